# revision 34
# baseline (speedup 1.0000x reference)
"""CIF layer (causal ConvTBC -> LN -> GELU -> Linear -> sigmoid -> CIF scatter)
as a Bass/Tile kernel for 8 Trainium2 NeuronCores, data-parallel over batch.

Key algorithmic reformulation: the CIF scatter is exactly an interval-overlap
weighting
    W[s, t] = clamp01(csum[s] - t) - clamp01(csum[s-1] - t)
so the whole scatter becomes a dense matmul  out[t, c] = sum_s W[s, t] x[s, c].
This is continuous in csum (no floor() cliffs), so f32 accumulation-order
differences vs the reference only produce O(eps)-weight shifts between
adjacent buckets.

Per-core work (2 batches): conv as 12 accumulating matmuls per 128-row chunk
(K=3 taps x 4 C-chunks), LN via bn_stats, GELU + sigmoid on ACT, cumsum via an
upper-triangular matmul + tiny free-dim scan, scatter as 64 accumulating
matmuls. Matmuls run in float32r (full PE rate) except the cumsum (exact f32).
"""

import numpy as np

import concourse.bacc as bacc
import concourse.bass as bass
import concourse.tile as tile
from concourse import mybir
from concourse.bass_utils import run_bass_kernel_spmd

# Problem dims (hardcoded per contract)
S, B, C, H, K = 2048, 16, 512, 512, 3
T_MAX = 512
BETA = 1.0
EPS_CIF = 1e-4
EPS_LN = 1e-5
N_CORES = 8
B_LOC = B // N_CORES          # batches per core
P = 128
M_CHUNKS = S // P             # 16 sequence chunks
C_CHUNKS = C // P             # 4 contraction chunks
T_CHUNKS = T_MAX // P         # 4 output-time chunks
SP = S + K - 1                # padded seq len for conv lhsT

F32 = mybir.dt.float32
AF = mybir.ActivationFunctionType
OP = mybir.AluOpType

_prog_cache: dict = {}


def _bcast_ap(ap, parts):
    """Partition-broadcast a [1, F...] AP to [parts, F...] via stride-0."""
    return bass.AP(tensor=ap.tensor, offset=ap.offset, ap=[[0, parts]] + ap.ap[1:])


def _build(flags):
    """Build + compile the SPMD program.
    flags = (mm_r, convb, lng, lnb, pmask, lin_b, gelu_mode)."""
    import os
    phases = int(os.environ.get("KM_PHASES", "3"))
    mm_r, has_convb, has_lng, has_lnb, has_pmask, lin_b_val, gelu_mode = flags
    mm_dt = mybir.dt.float32r if mm_r else mybir.dt.float32

    nc = bacc.Bacc("TRN2", target_bir_lowering=False, debug=False,
                   num_devices=N_CORES)

    # ---- DRAM I/O ----
    xt_pad = nc.dram_tensor("xt_pad", [B_LOC, C, SP], F32, kind="ExternalInput")
    xneg = nc.dram_tensor("xneg", [B_LOC, M_CHUNKS, P, C], F32, kind="ExternalInput")
    wconv = nc.dram_tensor("wconv", [K, C_CHUNKS, P, H], F32, kind="ExternalInput")
    linw = nc.dram_tensor("linw", [1, H], F32, kind="ExternalInput")
    trid = nc.dram_tensor("trid", [P, P], F32, kind="ExternalInput")
    negio = nc.dram_tensor("negio", [1, T_MAX], F32, kind="ExternalInput")
    desired = nc.dram_tensor("desired", [B_LOC, 1], F32, kind="ExternalInput")
    if has_convb:
        convb = nc.dram_tensor("convb", [1, H], F32, kind="ExternalInput")
    if has_lng:
        lngt = nc.dram_tensor("lngt", [1, H], F32, kind="ExternalInput")
    if has_lnb:
        lnbt = nc.dram_tensor("lnbt", [1, H], F32, kind="ExternalInput")
    if has_pmask:
        pmaskt = nc.dram_tensor("pmaskt", [B_LOC, P, M_CHUNKS], F32, kind="ExternalInput")

    cif_out = nc.dram_tensor("cif_out", [B_LOC, T_CHUNKS, P, C], F32, kind="ExternalOutput")
    alpha_out = nc.dram_tensor("alpha_out", [B_LOC, P, M_CHUNKS], F32, kind="ExternalOutput")

    with tile.TileContext(nc) as tc:
        from contextlib import ExitStack
        with ExitStack() as ctx:
            consts = ctx.enter_context(tc.tile_pool(name="consts", bufs=1))
            xt_pool = ctx.enter_context(tc.tile_pool(name="xt", bufs=2))
            h_pool = ctx.enter_context(tc.tile_pool(name="h", bufs=3))
            stat_pool = ctx.enter_context(tc.tile_pool(name="stat", bufs=4))
            wm_pool = ctx.enter_context(tc.tile_pool(name="wm", bufs=3))
            xn_pool = ctx.enter_context(tc.tile_pool(name="xn", bufs=3))
            out_pool = ctx.enter_context(tc.tile_pool(name="outp", bufs=4))
            small = ctx.enter_context(tc.tile_pool(name="small", bufs=2))
            psum_h = ctx.enter_context(tc.tile_pool(name="psum_h", bufs=2, space="PSUM"))
            psum_sc = ctx.enter_context(tc.tile_pool(name="psum_sc", bufs=1, space="PSUM"))
            psum_tri = ctx.enter_context(tc.tile_pool(name="psum_tri", bufs=1, space="PSUM"))

            # ---- constants (loaded once) ----
            # matmul operand tiles are declared in mm_dt (fp32r for full PE
            # rate); fp32<->fp32r is a bitcast-identical storage format, so
            # the DRAM side is bitcast at DMA time.
            wt = consts.tile([P, K, C_CHUNKS, H], mm_dt)
            for k in range(K):
                for c in range(C_CHUNKS):
                    nc.sync.dma_start(out=wt[:, k, c, :],
                                      in_=wconv.ap()[k, c].bitcast(mm_dt))
            linw_sb = consts.tile([P, H], F32)
            nc.sync.dma_start(out=linw_sb, in_=_bcast_ap(linw.ap(), P))
            negio_sb = consts.tile([P, T_MAX], F32)
            nc.sync.dma_start(out=negio_sb, in_=_bcast_ap(negio.ap(), P))
            tri_sb = consts.tile([P, P], F32)
            nc.sync.dma_start(out=tri_sb, in_=trid.ap())
            eps_col = consts.tile([P, 1], F32)
            nc.vector.memset(eps_col, EPS_LN)
            zcol = consts.tile([P, 1], F32)
            nc.vector.memset(zcol, 0.0)
            linb_col = consts.tile([P, 1], F32)
            nc.vector.memset(linb_col, float(lin_b_val))
            ones_row = consts.tile([1, P], F32, name="ones_row")
            nc.vector.memset(ones_row, 1.0)
            ones_sq = consts.tile([P, P], F32, name="ones_sq")
            nc.vector.memset(ones_sq, 1.0)
            if has_convb:
                convb_sb = consts.tile([1, H], F32)
                nc.sync.dma_start(out=convb_sb, in_=convb.ap())
            if has_lng:
                lng_sb = consts.tile([P, H], F32)
                nc.sync.dma_start(out=lng_sb, in_=_bcast_ap(lngt.ap(), P))
            if has_lnb:
                lnb_sb = consts.tile([P, H], F32)
                nc.sync.dma_start(out=lnb_sb, in_=_bcast_ap(lnbt.ap(), P))

            for b in range(B_LOC):
                # ======== phase 1: conv -> LN -> GELU -> dot -> sigmoid ========
                xt_sb = xt_pool.tile([P, C_CHUNKS, SP], mm_dt)
                for c in range(C_CHUNKS):
                    nc.sync.dma_start(
                        out=xt_sb[:, c, :],
                        in_=xt_pad.ap()[b, c * P:(c + 1) * P, :].bitcast(mm_dt))
                alpha_mat = small.tile([P, M_CHUNKS], F32, tag="alpha")

                for m in range(M_CHUNKS):
                    h_ps = psum_h.tile([P, H], F32)
                    n_acc = K * C_CHUNKS + (1 if has_convb else 0)
                    i_acc = 0
                    if has_convb:
                        nc.tensor.matmul(h_ps, lhsT=ones_row, rhs=convb_sb,
                                         start=True, stop=(n_acc == 1))
                        i_acc = 1
                    for c in range(C_CHUNKS):
                        for k in range(K):
                            lhs = xt_sb[:, c, m * P + k: m * P + k + P]
                            nc.tensor.matmul(
                                h_ps, lhsT=lhs, rhs=wt[:, k, c, :],
                                start=(i_acc == 0), stop=(i_acc == n_acc - 1))
                            i_acc += 1

                    p1sub = int(os.environ.get("KM_P1SUB", "4"))
                    if p1sub < 2:
                        # conv + PSUM->SBUF reduce only
                        nc.vector.reduce_sum(out=alpha_mat[:, m:m + 1],
                                             in_=h_ps, axis=mybir.AxisListType.X)
                        continue
                    stats = stat_pool.tile([P, 6], F32)
                    nc.vector.bn_stats(out=stats, in_=h_ps)
                    mv = stat_pool.tile([P, 2], F32)
                    nc.vector.bn_aggr(out=mv, in_=stats)
                    std = stat_pool.tile([P, 1], F32)
                    nc.scalar.activation(out=std, in_=mv[:, 1:2], func=AF.Sqrt,
                                         bias=eps_col, scale=1.0)
                    rstd = stat_pool.tile([P, 1], F32)
                    nc.vector.reciprocal(out=rstd, in_=std)
                    normed = h_pool.tile([P, H], F32)
                    nc.vector.tensor_scalar(out=normed, in0=h_ps,
                                            scalar1=mv[:, 0:1], scalar2=rstd,
                                            op0=OP.subtract, op1=OP.mult)
                    if has_lng:
                        nc.vector.tensor_mul(out=normed, in0=normed, in1=lng_sb)
                    if has_lnb:
                        nc.vector.tensor_add(out=normed, in0=normed, in1=lnb_sb)
                    if p1sub < 3:
                        nc.vector.reduce_sum(out=alpha_mat[:, m:m + 1],
                                             in_=normed, axis=mybir.AxisListType.X)
                        continue
                    gel = h_pool.tile([P, H], F32)
                    if gelu_mode == "hw":
                        nc.scalar.activation(out=gel, in_=normed, func=AF.Gelu,
                                             bias=zcol)
                    else:
                        # tanh-approx gelu from sim-supported primitives
                        # (CoreSim-only path; HW uses the exact Gelu table)
                        x2 = h_pool.tile([P, H], F32, tag="gx2")
                        nc.scalar.activation(out=x2, in_=normed, func=AF.Square,
                                             bias=zcol)
                        t1 = h_pool.tile([P, H], F32, tag="gt1")
                        nc.vector.tensor_scalar(out=t1, in0=x2, scalar1=0.044715,
                                                scalar2=1.0, op0=OP.mult, op1=OP.add)
                        u = h_pool.tile([P, H], F32, tag="gu")
                        nc.vector.tensor_mul(out=u, in0=t1, in1=normed)
                        th = h_pool.tile([P, H], F32, tag="gth")
                        nc.scalar.activation(out=th, in_=u, func=AF.Tanh,
                                             bias=zcol, scale=0.7978845608028654)
                        t3 = h_pool.tile([P, H], F32, tag="gt3")
                        nc.vector.tensor_scalar(out=t3, in0=th, scalar1=1.0,
                                                scalar2=0.5, op0=OP.add, op1=OP.mult)
                        nc.vector.tensor_mul(out=gel, in0=t3, in1=normed)
                    if p1sub < 4:
                        nc.vector.reduce_sum(out=alpha_mat[:, m:m + 1],
                                             in_=gel, axis=mybir.AxisListType.X)
                        continue
                    logit = stat_pool.tile([P, 1], F32)
                    if os.environ.get("KM_TTR"):
                        nc.vector.tensor_tensor_reduce(
                            out=gel, in0=gel, in1=linw_sb, scale=1.0,
                            scalar=float(lin_b_val), op0=OP.mult, op1=OP.add,
                            accum_out=logit)
                    else:
                        gl = h_pool.tile([P, H], F32, tag="gl")
                        nc.vector.scalar_tensor_tensor(
                            out=gl, in0=gel, scalar=0.0,
                            in1=linw_sb, op0=OP.add, op1=OP.mult,
                            accum_out=logit)
                    nc.scalar.activation(out=alpha_mat[:, m:m + 1], in_=logit,
                                         func=AF.Sigmoid, bias=linb_col)

                if has_pmask:
                    pm_sb = small.tile([P, M_CHUNKS], F32, tag="pmask")
                    nc.sync.dma_start(out=pm_sb, in_=pmaskt.ap()[b])
                    nc.vector.tensor_mul(out=alpha_mat, in0=alpha_mat, in1=pm_sb)

                nc.sync.dma_start(out=alpha_out.ap()[b], in_=alpha_mat)
                if phases < 2:
                    continue

                # ======== phase 2: cumsum + scale ========
                # one PSUM bank holds: [0:16] per-chunk local csums,
                # [16:32] chunk totals replicated to all partitions (row 0 is
                # engine-addressable; row 127 of the tri result is not),
                # [32:49] the broadcast offsets+scale row
                tri_ps = psum_tri.tile([P, 64], F32)
                nc.tensor.matmul(tri_ps[:, 0:M_CHUNKS], lhsT=tri_sb,
                                 rhs=alpha_mat, start=True, stop=True)
                nc.tensor.matmul(tri_ps[:, 16:16 + M_CHUNKS], lhsT=ones_sq,
                                 rhs=alpha_mat, start=True, stop=True)
                cs_row = tri_ps[0:1, 16:16 + M_CHUNKS]
                zrow = small.tile([1, M_CHUNKS], F32, tag="zrow")
                nc.vector.memset(zrow, 0.0)
                incl = small.tile([1, M_CHUNKS], F32, tag="incl")
                nc.vector.tensor_tensor_scan(
                    out=incl, data0=cs_row, data1=zrow,
                    initial=0.0, op0=OP.add, op1=OP.add)
                des_sb = small.tile([1, 1], F32, tag="des")
                nc.sync.dma_start(out=des_sb, in_=desired.ap()[b:b + 1, :])
                pk = small.tile([1, 18], F32, tag="pk")
                # pk[0:16] = exclusive chunk offsets, pk[16] = scale
                nc.vector.tensor_sub(out=pk[:, 0:M_CHUNKS], in0=incl,
                                     in1=cs_row)
                recip = small.tile([1, 1], F32, tag="recip")
                nc.vector.reciprocal(out=recip, in_=incl[:, M_CHUNKS - 1:M_CHUNKS])
                nc.vector.tensor_mul(out=pk[:, 16:17], in0=recip, in1=des_sb)
                # broadcast pk to all partitions with a K=1 matmul (race-free,
                # no DRAM roundtrip)
                nc.tensor.matmul(tri_ps[:, 32:49], lhsT=ones_row,
                                 rhs=pk[:, 0:17], start=True, stop=True)
                bc = small.tile([P, 17], F32, tag="bc")
                nc.scalar.copy(out=bc, in_=tri_ps[:, 32:49])

                tmp_cs = small.tile([P, M_CHUNKS], F32, tag="tmp_cs")
                nc.vector.tensor_tensor(out=tmp_cs, in0=tri_ps[:, 0:M_CHUNKS],
                                        in1=bc[:, 0:M_CHUNKS], op=OP.add)
                cs_s = small.tile([P, M_CHUNKS], F32, tag="cs_s")
                nc.vector.tensor_scalar_mul(out=cs_s, in0=tmp_cs,
                                            scalar1=bc[:, 16:17])
                p_raw = small.tile([P, M_CHUNKS], F32, tag="p_raw")
                nc.vector.tensor_sub(out=p_raw, in0=tmp_cs, in1=alpha_mat)
                ps_s = small.tile([P, M_CHUNKS], F32, tag="ps_s")
                nc.vector.tensor_scalar_mul(out=ps_s, in0=p_raw,
                                            scalar1=bc[:, 16:17])

                if phases < 3:
                    continue
                # ======== phase 3: weight build + scatter matmul ========
                out_ps = [psum_sc.tile([P, C], F32, tag=f"sc{t}", name=f"sc{t}")
                          for t in range(T_CHUNKS)]
                for m in range(M_CHUNKS):
                    c_col = cs_s[:, m:m + 1]
                    p_col = ps_s[:, m:m + 1]
                    # A = relu(c - t), Bt = relu(p - t) on ACT (negio = -t)
                    A = wm_pool.tile([P, T_MAX], F32, tag="A")
                    nc.scalar.activation(out=A, in_=negio_sb, func=AF.Relu,
                                         bias=c_col, scale=1.0)
                    Bt = wm_pool.tile([P, T_MAX], F32, tag="B")
                    nc.scalar.activation(out=Bt, in_=negio_sb, func=AF.Relu,
                                         bias=p_col, scale=1.0)
                    Am = wm_pool.tile([P, T_MAX], F32, tag="Am")
                    nc.vector.tensor_scalar_min(out=Am, in0=A, scalar1=1.0)
                    # wneg = min(Bt,1) - min(A,1) = -W  (paired with negated x)
                    wneg = wm_pool.tile([P, T_MAX], mm_dt, tag="wneg")
                    nc.vector.scalar_tensor_tensor(
                        out=wneg, in0=Bt, scalar=1.0, in1=Am,
                        op0=OP.min, op1=OP.subtract)
                    xn_t = xn_pool.tile([P, C], mm_dt)
                    nc.sync.dma_start(out=xn_t,
                                      in_=xneg.ap()[b, m].bitcast(mm_dt))
                    for t in range(T_CHUNKS):
                        nc.tensor.matmul(
                            out_ps[t], lhsT=wneg[:, t * P:(t + 1) * P],
                            rhs=xn_t,
                            start=(m == 0), stop=(m == M_CHUNKS - 1))
                for t in range(T_CHUNKS):
                    ot = out_pool.tile([P, C], F32)
                    nc.scalar.copy(out=ot, in_=out_ps[t])
                    nc.sync.dma_start(out=cif_out.ap()[b, t], in_=ot)

    nc.compile()
    return nc


def _prep_inputs(x, conv_w, lin_w, target_lengths, conv_b, ln_g, ln_b,
                 padding_mask, flags):
    """Shard + lay out inputs for the 8 cores."""
    mm_r, has_convb, has_lng, has_lnb, has_pmask = flags[:5]
    wconv_np = np.ascontiguousarray(conv_w.reshape(K, C_CHUNKS, P, H))
    linw_np = np.ascontiguousarray(lin_w[:, 0][None, :])
    tri_np = np.triu(np.ones((P, P), np.float32))
    negio_np = -np.arange(T_MAX, dtype=np.float32)[None, :]

    in_maps = []
    for i in range(N_CORES):
        sel = slice(i * B_LOC, (i + 1) * B_LOC)
        xb = x[:, sel, :].transpose(1, 0, 2)              # (B_LOC, S, C)
        xt = np.zeros((B_LOC, C, SP), np.float32)
        xt[:, :, K - 1:] = xb.transpose(0, 2, 1)
        m = {
            "xt_pad": xt,
            "xneg": np.ascontiguousarray(-xb.reshape(B_LOC, M_CHUNKS, P, C)),
            "wconv": wconv_np,
            "linw": linw_np,
            "trid": tri_np,
            "negio": negio_np,
            "desired": (BETA * target_lengths[sel].astype(np.float64) + EPS_CIF)
                        .astype(np.float32).reshape(B_LOC, 1),
        }
        if has_convb:
            m["convb"] = np.ascontiguousarray(conv_b[None, :].astype(np.float32))
        if has_lng:
            m["lngt"] = np.ascontiguousarray(ln_g[None, :].astype(np.float32))
        if has_lnb:
            m["lnbt"] = np.ascontiguousarray(ln_b[None, :].astype(np.float32))
        if has_pmask:
            keep = (~padding_mask[sel]).astype(np.float32)        # (B_LOC, S)
            m["pmaskt"] = np.ascontiguousarray(
                keep.reshape(B_LOC, M_CHUNKS, P).transpose(0, 2, 1))
        in_maps.append(m)
    return in_maps


def _assemble(results):
    cif = np.empty((B, T_MAX, C), np.float32)
    alpha = np.empty((B, S), np.float32)
    for i in range(N_CORES):
        r = results[i]
        co = np.asarray(r["cif_out"]).reshape(B_LOC, T_MAX, C)
        ao = np.asarray(r["alpha_out"])                    # (B_LOC, P, M_CHUNKS)
        for bl in range(B_LOC):
            bg = i * B_LOC + bl
            cif[bg] = co[bl]
            alpha[bg] = ao[bl].T.reshape(S)
    return np.ascontiguousarray(cif.transpose(1, 0, 2)), alpha


def _get_prog(flags):
    if flags not in _prog_cache:
        _prog_cache[flags] = _build(flags)
    return _prog_cache[flags]


def kernel(x, conv_w, conv_b, ln_g, ln_b, lin_w, lin_b, padding_mask,
           target_lengths, _run=None, _mm_r=True, _gelu="hw"):
    x = np.asarray(x, np.float32)
    conv_w = np.asarray(conv_w, np.float32)
    conv_b = np.asarray(conv_b, np.float32)
    ln_g = np.asarray(ln_g, np.float32)
    ln_b = np.asarray(ln_b, np.float32)
    lin_w = np.asarray(lin_w, np.float32)
    lin_b = np.asarray(lin_b, np.float32)
    padding_mask = np.asarray(padding_mask)
    target_lengths = np.asarray(target_lengths)

    flags = (
        bool(_mm_r),
        bool(np.any(conv_b)),
        bool(np.any(ln_g != 1.0)),
        bool(np.any(ln_b)),
        bool(np.any(padding_mask)),
        float(lin_b[0]),
        _gelu,
    )
    nc = _get_prog(flags)
    in_maps = _prep_inputs(x, conv_w, lin_w, target_lengths, conv_b, ln_g,
                           ln_b, padding_mask, flags)
    if _run is not None:                     # test hook (e.g. CoreSim)
        results = _run(nc, in_maps)
    else:
        results = run_bass_kernel_spmd(nc, in_maps,
                                       core_ids=list(range(N_CORES))).results
    return _assemble(results)


# revision 58
# speedup vs baseline: 20317.5453x; 20317.5453x over previous
"""CIF layer (causal ConvTBC -> LN -> GELU -> Linear -> sigmoid -> CIF scatter)
as a Bass/Tile kernel for 8 Trainium2 NeuronCores, data-parallel over batch.

Key reformulation: the CIF scatter is exactly interval-overlap weighting
    W[s, t] = clamp01(csum[s] - t) - clamp01(csum[s-1] - t)
so the scatter becomes a dense matmul  out[t, c] = sum_s W[s, t] x[s, c],
continuous in csum (no floor() cliffs => f32-accumulation-order differences
vs the reference only shift O(eps) weight between adjacent buckets).

Engine layout (per core, 2 batches):
  PE   : conv = 12 accumulating fp32r matmuls per 128-row chunk; cumsum via
         triangular matmul; scatter = 64 accumulating fp32r matmuls.
  ACT  : PSUM->SBUF copies, LN apply (Identity w/ per-row scale+bias),
         exact gelu via Erf (same activation-function set as Relu/Sigmoid/
         Copy/Square => almost no 1.3us table reloads), phase-3 clamp relus.
  DVE  : bn_stats/bn_aggr LN stats, dot-products via scalar_tensor_tensor
         accumulate, weight-matrix combine.
  Pool : second clamp01 chain + gelu multiply (idle engine otherwise).
  DMA  : few large transfers (the SP sequencer is held for a DMA's whole
         duration, so batching transfers matters).
"""

import os
import numpy as np

import concourse.bacc as bacc
import concourse.bass as bass
import concourse.tile as tile
from concourse import mybir
from concourse.bass_utils import run_bass_kernel_spmd

# Problem dims (hardcoded per contract)
S, B, C, H, K = 2048, 16, 512, 512, 3
T_MAX = 512
BETA = 1.0
EPS_CIF = 1e-4
EPS_LN = 1e-5
N_CORES = 8
B_LOC = B // N_CORES          # batches per core
P = 128
M_CHUNKS = S // P             # 16 sequence chunks
C_CHUNKS = C // P             # 4 contraction chunks
T_CHUNKS = T_MAX // P         # 4 output-time chunks
SP = S + K - 1                # padded seq len for conv lhsT
INV_SQRT2 = 0.7071067811865476

F32 = mybir.dt.float32
AF = mybir.ActivationFunctionType
OP = mybir.AluOpType

_prog_cache: dict = {}


def _build(flags):
    """Build + compile the SPMD program.
    flags = (mm_r, convb, lng, lnb, pmask, lin_b, gelu_mode)."""
    mm_r, has_convb, has_lng, has_lnb, has_pmask, lin_b_val, gelu_mode = flags
    mm_dt = mybir.dt.float32r if mm_r else mybir.dt.float32
    phases = int(os.environ.get("KM_PHASES", "3"))

    nc = bacc.Bacc("TRN2", target_bir_lowering=False, debug=False,
                   num_devices=N_CORES)

    # ---- DRAM I/O ----
    xt_pad = nc.dram_tensor("xt_pad", [B_LOC, C, SP], F32, kind="ExternalInput")
    xneg = nc.dram_tensor("xneg", [B_LOC, M_CHUNKS, P, C], F32, kind="ExternalInput")
    wconv = nc.dram_tensor("wconv", [K, C_CHUNKS, P, H], F32, kind="ExternalInput")
    linw = nc.dram_tensor("linw", [1, H], F32, kind="ExternalInput")
    trid = nc.dram_tensor("trid", [P, P], F32, kind="ExternalInput")
    negio = nc.dram_tensor("negio", [1, T_MAX], F32, kind="ExternalInput")
    desired = nc.dram_tensor("desired", [B_LOC, 1], F32, kind="ExternalInput")
    if has_convb:
        convb = nc.dram_tensor("convb", [1, H], F32, kind="ExternalInput")
    if has_lng:
        lngt = nc.dram_tensor("lngt", [1, H], F32, kind="ExternalInput")
    if has_lnb:
        lnbt = nc.dram_tensor("lnbt", [1, H], F32, kind="ExternalInput")
    if has_pmask:
        pmaskt = nc.dram_tensor("pmaskt", [B_LOC, P, M_CHUNKS], F32, kind="ExternalInput")

    cif_out = nc.dram_tensor("cif_out", [B_LOC, T_CHUNKS, P, C], F32, kind="ExternalOutput")
    alpha_out = nc.dram_tensor("alpha_out", [B_LOC, P, M_CHUNKS], F32, kind="ExternalOutput")

    with tile.TileContext(nc) as tc:
        from contextlib import ExitStack
        with ExitStack() as ctx:
            consts = ctx.enter_context(tc.tile_pool(name="consts", bufs=1))
            xt_pool = ctx.enter_context(tc.tile_pool(name="xt", bufs=6))
            hall_pool = ctx.enter_context(tc.tile_pool(name="hall", bufs=2))
            stat_pool = ctx.enter_context(tc.tile_pool(name="stat", bufs=4))
            erf_pool = ctx.enter_context(tc.tile_pool(name="erf", bufs=2))
            gl_pool = ctx.enter_context(tc.tile_pool(name="gl", bufs=2))
            wm_pool = ctx.enter_context(tc.tile_pool(name="wm", bufs=2))
            xn_pool = ctx.enter_context(tc.tile_pool(name="xn", bufs=2))
            out_pool = ctx.enter_context(tc.tile_pool(name="outp", bufs=1))
            small = ctx.enter_context(tc.tile_pool(name="small", bufs=2))
            psum_h = ctx.enter_context(tc.tile_pool(name="psum_h", bufs=3, space="PSUM"))
            psum_sc = ctx.enter_context(tc.tile_pool(name="psum_sc", bufs=1, space="PSUM"))
            psum_tri = ctx.enter_context(tc.tile_pool(name="psum_tri", bufs=1, space="PSUM"))

            # ---- constants (loaded once; matmul operands typed mm_dt — an
            # identical bit layout to f32, so the DRAM side is just bitcast) ----
            wt = consts.tile([P, K, C_CHUNKS, H], mm_dt)
            wt_loaded = [False]

            def load_wt_c(c):
                nc.sync.dma_start(
                    out=wt[:, :, c:c + 1, :],
                    in_=wconv.ap()[:, c].rearrange("k p h -> p k h").bitcast(mm_dt))
            linw_sb = consts.tile([P, H], F32)   # pre-halved on host (gelu 0.5)
            lw = linw.ap()
            nc.sync.dma_start(out=linw_sb, in_=bass.AP(
                tensor=lw.tensor, offset=lw.offset, ap=[[0, P]] + lw.ap[1:]))
            negio_sb = consts.tile([P, T_MAX], F32)
            ng = negio.ap()
            nc.sync.dma_start(out=negio_sb, in_=bass.AP(
                tensor=ng.tensor, offset=ng.offset, ap=[[0, P]] + ng.ap[1:]))
            tri_sb = consts.tile([P, P], F32)
            nc.sync.dma_start(out=tri_sb, in_=trid.ap())
            eps_col = consts.tile([P, 1], F32)
            nc.vector.memset(eps_col, EPS_LN)
            zcol = consts.tile([P, 1], F32)
            nc.vector.memset(zcol, 0.0)
            one_col = consts.tile([P, 1], F32)
            nc.vector.memset(one_col, 1.0)
            linb_col = consts.tile([P, 1], F32)
            nc.vector.memset(linb_col, float(lin_b_val))
            ones_row = consts.tile([1, P], F32, name="ones_row")
            nc.vector.memset(ones_row, 1.0)
            ones_sq = consts.tile([P, P], F32, name="ones_sq")
            nc.vector.memset(ones_sq, 1.0)
            if has_convb:
                convb_sb = consts.tile([1, H], F32)
                nc.sync.dma_start(out=convb_sb, in_=convb.ap())
            if has_lng:
                lng_sb = consts.tile([P, H], F32)
                lg = lngt.ap()
                nc.sync.dma_start(out=lng_sb, in_=bass.AP(
                    tensor=lg.tensor, offset=lg.offset, ap=[[0, P]] + lg.ap[1:]))
            if has_lnb:
                lnb_sb = consts.tile([P, H], F32)
                lb = lnbt.ap()
                nc.sync.dma_start(out=lnb_sb, in_=bass.AP(
                    tensor=lb.tensor, offset=lb.offset, ap=[[0, P]] + lb.ap[1:]))

            state = [dict() for _ in range(B_LOC)]

            def emit_A(b):
                st = state[b]
                xt_cs = []
                for c in range(C_CHUNKS):
                    if not wt_loaded[0]:
                        load_wt_c(c)          # interleave with xt for fast start
                    xc = xt_pool.tile([P, SP], mm_dt, tag="xtc", name=f"xt{c}")
                    nc.sync.dma_start(
                        out=xc,
                        in_=xt_pad.ap()[b, c * P:(c + 1) * P, :].bitcast(mm_dt))
                    xt_cs.append(xc)
                if not wt_loaded[0]:
                    wt_loaded[0] = True
                logit_mat = small.tile([P, M_CHUNKS], F32, tag="logit",
                                       name="logit")
                sums_mat = small.tile([P, M_CHUNKS], F32, tag="sums_mat",
                                      name="sums_mat")
                sumsq_mat = small.tile([P, M_CHUNKS], F32, tag="sumsq_mat",
                                       name="sumsq_mat")
                h_all = hall_pool.tile([P, M_CHUNKS, H], F32)
                st.update(logit_mat=logit_mat, sums_mat=sums_mat,
                          sumsq_mat=sumsq_mat, h_all=h_all)
                for m in range(M_CHUNKS):
                    h_ps = psum_h.tile([P, H], F32, tag="h_ps", name="h_ps")
                    n_acc = K * C_CHUNKS + (1 if has_convb else 0)
                    i_acc = 0
                    if has_convb:
                        nc.tensor.matmul(h_ps, lhsT=ones_row, rhs=convb_sb,
                                         start=True, stop=(n_acc == 1))
                        i_acc = 1
                    for c in range(C_CHUNKS):
                        for k in range(K):
                            lhs = xt_cs[c][:, m * P + k: m * P + k + P]
                            nc.tensor.matmul(
                                h_ps, lhsT=lhs, rhs=wt[:, k, c, :],
                                start=(i_acc == 0), stop=(i_acc == n_acc - 1))
                            i_acc += 1
                    # copy h -> SBUF with row-sum accum; row-sum-of-squares
                    nc.scalar.activation(out=h_all[:, m, :], in_=h_ps,
                                         func=AF.Identity, bias=zcol,
                                         accum_out=sums_mat[:, m:m + 1])
                    sqd = gl_pool.tile([P, H], F32, tag="sqd", name="sqd")
                    nc.vector.scalar_tensor_tensor(
                        out=sqd, in0=h_all[:, m, :], scalar=1.0,
                        in1=h_all[:, m, :], op0=OP.mult, op1=OP.mult,
                        accum_out=sumsq_mat[:, m:m + 1])

            def emit_B(b):
                st = state[b]
                i32 = mybir.dt.int32
                erf_scale = INV_SQRT2 if gelu_mode == "hw" else 1.0
                sums_mat = st["sums_mat"]
                sumsq_mat = st["sumsq_mat"]
                h_all = st["h_all"]
                logit_mat = st["logit_mat"]
                # batched LN stats + quake/Newton rsqrt on [P,16]
                nmu_mat = small.tile([P, M_CHUNKS], F32, tag="nmu_mat",
                                     name="nmu_mat")
                rstd_mat = small.tile([P, M_CHUNKS], F32, tag="rstd_mat",
                                      name="rstd_mat")
                mug = small.tile([P, M_CHUNKS], F32, tag="mug", name="mug")
                nc.vector.tensor_scalar_mul(out=mug, in0=sums_mat, scalar1=1.0 / H)
                nc.vector.tensor_scalar_mul(out=nmu_mat, in0=sums_mat,
                                            scalar1=-1.0 / H)
                sqe = small.tile([P, M_CHUNKS], F32, tag="sqe", name="sqe")
                nc.vector.tensor_scalar(out=sqe, in0=sumsq_mat, scalar1=1.0 / H,
                                        scalar2=EPS_LN, op0=OP.mult, op1=OP.add)
                mu2 = small.tile([P, M_CHUNKS], F32, tag="mu2", name="mu2")
                nc.vector.scalar_tensor_tensor(out=mu2, in0=mug, scalar=1.0,
                                               in1=mug, op0=OP.mult, op1=OP.mult)
                vt = small.tile([P, M_CHUNKS], F32, tag="vt", name="vt")
                nc.vector.scalar_tensor_tensor(out=vt, in0=sqe, scalar=1.0,
                                               in1=mu2, op0=OP.mult,
                                               op1=OP.subtract)  # var+eps
                sh = small.tile([P, M_CHUNKS], i32, tag="sh", name="sh")
                nc.vector.tensor_scalar(out=sh, in0=vt.bitcast(i32),
                                        scalar1=1, scalar2=None,
                                        op0=OP.arith_shift_right)
                y0i = small.tile([P, M_CHUNKS], i32, tag="y0i", name="y0i")
                nc.vector.tensor_scalar(out=y0i, in0=sh, scalar1=0x5F3759DF,
                                        scalar2=-1, op0=OP.subtract, op1=OP.mult)
                y0 = y0i.bitcast(F32)
                r = small.tile([P, M_CHUNKS], F32, tag="r", name="r")
                nc.vector.scalar_tensor_tensor(out=r, in0=y0, scalar=1.0,
                                               in1=y0, op0=OP.mult, op1=OP.mult)
                r2 = small.tile([P, M_CHUNKS], F32, tag="r2", name="r2")
                nc.vector.scalar_tensor_tensor(out=r2, in0=r, scalar=1.0,
                                               in1=vt, op0=OP.mult, op1=OP.mult)
                f1 = small.tile([P, M_CHUNKS], F32, tag="f1", name="f1")
                nc.vector.tensor_scalar(out=f1, in0=r2, scalar1=-0.5,
                                        scalar2=1.5, op0=OP.mult, op1=OP.add)
                y1 = small.tile([P, M_CHUNKS], F32, tag="y1", name="y1")
                nc.vector.scalar_tensor_tensor(out=y1, in0=y0, scalar=1.0,
                                               in1=f1, op0=OP.mult, op1=OP.mult)
                rb = small.tile([P, M_CHUNKS], F32, tag="rb", name="rb")
                nc.vector.scalar_tensor_tensor(out=rb, in0=y1, scalar=1.0,
                                               in1=y1, op0=OP.mult, op1=OP.mult)
                rb2 = small.tile([P, M_CHUNKS], F32, tag="rb2", name="rb2")
                nc.vector.scalar_tensor_tensor(out=rb2, in0=rb, scalar=1.0,
                                               in1=vt, op0=OP.mult, op1=OP.mult)
                f2 = small.tile([P, M_CHUNKS], F32, tag="f2", name="f2")
                nc.vector.tensor_scalar(out=f2, in0=rb2, scalar1=-0.5,
                                        scalar2=1.5, op0=OP.mult, op1=OP.add)
                nc.vector.scalar_tensor_tensor(
                    out=rstd_mat, in0=y1, scalar=erf_scale, in1=f2,
                    op0=OP.mult, op1=OP.mult)

                st["nmu_mat"] = nmu_mat
                st["rstd_mat"] = rstd_mat

            def emit_B_chunks(b, ms):
                st = state[b]
                nmu_mat = st["nmu_mat"]
                rstd_mat = st["rstd_mat"]
                h_all = st["h_all"]
                logit_mat = st["logit_mat"]
                # LN apply + gelu + dot per chunk (no PE)
                for m in ms:
                    hs = h_all[:, m, :]
                    nmu = nmu_mat[:, m:m + 1]
                    rstd_x = rstd_mat[:, m:m + 1]
                    nc.scalar.activation(out=hs, in_=hs, func=AF.Identity,
                                         bias=nmu, scale=1.0)   # centered
                    if has_lng or has_lnb:
                        nc.vector.tensor_scalar_mul(out=hs, in0=hs,
                                                    scalar1=rstd_x)
                        nc.vector.tensor_scalar_mul(out=hs, in0=hs,
                                                    scalar1=1.0 / erf_scale)
                        if has_lng:
                            nc.vector.tensor_mul(out=hs, in0=hs, in1=lng_sb)
                        if has_lnb:
                            nc.vector.tensor_add(out=hs, in0=hs, in1=lnb_sb)
                    if gelu_mode == "hw":
                        # erf(cen * rstd/sqrt2); gelu's 0.5 and the sqrt2
                        # compensation live in host-side linw
                        erf_c = erf_pool.tile([P, H], F32, tag="erf", name="erf")
                        nc.scalar.activation(
                            out=erf_c, in_=hs, func=AF.Erf, bias=zcol,
                            scale=INV_SQRT2 if (has_lng or has_lnb) else rstd_x)
                    else:
                        # tanh-approx gelu (CoreSim path)
                        if not (has_lng or has_lnb):
                            nc.vector.tensor_scalar_mul(out=hs, in0=hs,
                                                        scalar1=rstd_x)
                        x2 = erf_pool.tile([P, H], F32, tag="erf", name="x2")
                        nc.scalar.activation(out=x2, in_=hs, func=AF.Square,
                                             bias=zcol)
                        t1 = erf_pool.tile([P, H], F32, tag="erf", name="t1")
                        nc.vector.tensor_scalar(out=t1, in0=x2, scalar1=0.044715,
                                                scalar2=1.0, op0=OP.mult, op1=OP.add)
                        u = erf_pool.tile([P, H], F32, tag="erf", name="u")
                        nc.vector.tensor_mul(out=u, in0=t1, in1=hs)
                        erf_c = erf_pool.tile([P, H], F32, tag="erf", name="erf")
                        nc.scalar.activation(out=erf_c, in_=u, func=AF.Tanh,
                                             bias=zcol, scale=0.7978845608028654)
                    nc.gpsimd.tensor_scalar_add(out=erf_c, in0=erf_c,
                                                scalar1=1.0)
                    nc.gpsimd.tensor_mul(out=hs, in0=erf_c, in1=hs)
                    gl = gl_pool.tile([P, H], F32, tag="gl", name="gl")
                    if gelu_mode == "hw":
                        dscalar = INV_SQRT2 if (has_lng or has_lnb) else rstd_x
                    else:
                        dscalar = 1.0
                    nc.vector.scalar_tensor_tensor(
                        out=gl, in0=hs, scalar=dscalar, in1=linw_sb,
                        op0=OP.mult, op1=OP.mult,
                        accum_out=logit_mat[:, m:m + 1])


            def emit_B_fin(b):
                st = state[b]
                logit_mat = st["logit_mat"]
                alpha_mat = small.tile([P, M_CHUNKS], F32, tag="alpha",
                                       name="alpha")
                nc.scalar.activation(out=alpha_mat, in_=logit_mat,
                                     func=AF.Sigmoid, bias=linb_col)
                if has_pmask:
                    pm_sb = small.tile([P, M_CHUNKS], F32, tag="pmask",
                                       name="pmask")
                    nc.sync.dma_start(out=pm_sb, in_=pmaskt.ap()[b])
                    nc.vector.tensor_mul(out=alpha_mat, in0=alpha_mat,
                                         in1=pm_sb)
                nc.sync.dma_start(out=alpha_out.ap()[b], in_=alpha_mat)
                st["alpha_mat"] = alpha_mat

            def emit_ph2(b):
                st = state[b]
                alpha_mat = st["alpha_mat"]
                # one PSUM bank: [0:16] local csums | [16:32] chunk totals
                # (replicated; row 0 is engine-addressable) | [32:49] broadcast
                tri_ps = psum_tri.tile([P, 64], F32, tag="tri_ps",
                                       name="tri_ps")
                nc.tensor.matmul(tri_ps[:, 0:M_CHUNKS], lhsT=tri_sb,
                                 rhs=alpha_mat, start=True, stop=True)
                nc.tensor.matmul(tri_ps[:, 16:16 + M_CHUNKS], lhsT=ones_sq,
                                 rhs=alpha_mat, start=True, stop=True)
                cs_row = tri_ps[0:1, 16:16 + M_CHUNKS]
                zrow = small.tile([1, M_CHUNKS], F32, tag="zrow", name="zrow")
                nc.vector.memset(zrow, 0.0)
                incl = small.tile([1, M_CHUNKS], F32, tag="incl", name="incl")
                nc.vector.tensor_tensor_scan(
                    out=incl, data0=cs_row, data1=zrow,
                    initial=0.0, op0=OP.add, op1=OP.add)
                des_sb = small.tile([1, 1], F32, tag="des", name="des")
                nc.sync.dma_start(out=des_sb, in_=desired.ap()[b:b + 1, :])
                pk = small.tile([1, 18], F32, tag="pk", name="pk")
                nc.vector.tensor_sub(out=pk[:, 0:M_CHUNKS], in0=incl,
                                     in1=cs_row)
                recip = small.tile([1, 1], F32, tag="recip", name="recip")
                nc.vector.reciprocal(out=recip,
                                     in_=incl[:, M_CHUNKS - 1:M_CHUNKS])
                nc.vector.tensor_mul(out=pk[:, 16:17], in0=recip, in1=des_sb)
                nc.tensor.matmul(tri_ps[:, 32:49], lhsT=ones_row,
                                 rhs=pk[:, 0:17], start=True, stop=True)
                bc = small.tile([P, 17], F32, tag="bc", name="bc")
                nc.scalar.copy(out=bc, in_=tri_ps[:, 32:49])

                tmp_cs = small.tile([P, M_CHUNKS], F32, tag="tmp_cs",
                                    name="tmp_cs")
                nc.vector.tensor_tensor(out=tmp_cs, in0=tri_ps[:, 0:M_CHUNKS],
                                        in1=bc[:, 0:M_CHUNKS], op=OP.add)
                cs_s = small.tile([P, M_CHUNKS], F32, tag="cs_s", name="cs_s")
                nc.vector.tensor_scalar_mul(out=cs_s, in0=tmp_cs,
                                            scalar1=bc[:, 16:17])
                p_raw = small.tile([P, M_CHUNKS], F32, tag="p_raw",
                                   name="p_raw")
                nc.vector.tensor_sub(out=p_raw, in0=tmp_cs, in1=alpha_mat)
                ps_s = small.tile([P, M_CHUNKS], F32, tag="ps_s", name="ps_s")
                nc.vector.tensor_scalar_mul(out=ps_s, in0=p_raw,
                                            scalar1=bc[:, 16:17])
                st.update(cs_s=cs_s, ps_s=ps_s)

            def emit_ph3_pre(b):
                st = state[b]
                xn_h = []
                for i in range(4):
                    xnt = xn_pool.tile([P, 4, C], mm_dt, tag="xnq",
                                       name=f"xn{i}")
                    nc.sync.dma_start(
                        out=xnt,
                        in_=xneg.ap()[b, i * 4:(i + 1) * 4]
                            .rearrange("m p c -> p m c").bitcast(mm_dt))
                    xn_h.append(xnt)
                st["xn_h"] = xn_h
                st["out_ps"] = [psum_sc.tile([P, C], F32, tag=f"sc{t}",
                                             name=f"sc{t}")
                                for t in range(T_CHUNKS)]

            def emit_ph3_chunks(b, ms):
                st = state[b]
                cs_s, ps_s = st["cs_s"], st["ps_s"]
                xn_h, out_ps = st["xn_h"], st["out_ps"]
                for m in ms:
                    c_col = cs_s[:, m:m + 1]
                    p_col = ps_s[:, m:m + 1]
                    # W[s,t] = max(0, min(c-t, 1) - max(p-t, 0));
                    # wneg = -W = min(m2 - m1, 0)  (4 ops, Pool+DVE only)
                    m1 = wm_pool.tile([P, T_MAX], F32, tag="m1", name="m1")
                    nc.gpsimd.tensor_scalar(out=m1, in0=negio_sb,
                                            scalar1=c_col, scalar2=1.0,
                                            op0=OP.add, op1=OP.min)
                    m2 = wm_pool.tile([P, T_MAX], F32, tag="m2", name="m2")
                    nc.scalar.activation(out=m2, in_=negio_sb, func=AF.Relu,
                                         bias=p_col, scale=1.0)
                    d = wm_pool.tile([P, T_MAX], F32, tag="d", name="d")
                    nc.vector.scalar_tensor_tensor(
                        out=d, in0=m2, scalar=1.0, in1=m1,
                        op0=OP.mult, op1=OP.subtract)
                    wneg = wm_pool.tile([P, T_MAX], mm_dt, tag="wneg",
                                        name="wneg")
                    nc.vector.tensor_scalar_min(out=wneg, in0=d, scalar1=0.0)
                    xn_t = xn_h[m // 4][:, m % 4, :]
                    for t in range(T_CHUNKS):
                        nc.tensor.matmul(
                            out_ps[t], lhsT=wneg[:, t * P:(t + 1) * P],
                            rhs=xn_t,
                            start=(m == 0), stop=(m == M_CHUNKS - 1))

            def emit_ph3_post(b):
                st = state[b]
                out_ps = st["out_ps"]
                out_all = out_pool.tile([P, T_CHUNKS, C], F32, tag="out_all",
                                        name="out_all")
                for t in range(T_CHUNKS):
                    nc.scalar.copy(out=out_all[:, t, :], in_=out_ps[t])
                nc.sync.dma_start(
                    out=cif_out.ap()[b].rearrange("t p c -> p t c"),
                    in_=out_all)

            # interleave batch phases so PE always has work:
            emit_A(0)
            emit_A(1)
            emit_B(0)
            emit_B_chunks(0, range(M_CHUNKS))
            emit_B_fin(0)
            if phases >= 2:
                emit_ph2(0)
            emit_B(1)
            if phases >= 3:
                emit_ph3_pre(0)
                for m in range(M_CHUNKS):
                    emit_ph3_chunks(0, [m])
                    emit_B_chunks(1, [m])
                emit_ph3_post(0)
            else:
                emit_B_chunks(1, range(M_CHUNKS))
            emit_B_fin(1)
            if phases >= 2:
                emit_ph2(1)
            if phases >= 3:
                emit_ph3_pre(1)
                emit_ph3_chunks(1, range(M_CHUNKS))
                emit_ph3_post(1)

    nc.compile()
    return nc


def _prep_inputs(x, conv_w, lin_w, target_lengths, conv_b, ln_g, ln_b,
                 padding_mask, flags):
    """Shard + lay out inputs for the 8 cores."""
    has_convb, has_lng, has_lnb, has_pmask = flags[1:5]
    wconv_np = np.ascontiguousarray(conv_w.reshape(K, C_CHUNKS, P, H))
    # gelu 0.5 fold; erf path additionally compensates the rstd_x = rstd/sqrt2
    # fold (0.7071*0.7071 = 0.5)
    lw_scale = INV_SQRT2 if flags[6] == "hw" else 0.5
    linw_np = np.ascontiguousarray(lw_scale * lin_w[:, 0][None, :])
    tri_np = np.triu(np.ones((P, P), np.float32))
    negio_np = -np.arange(T_MAX, dtype=np.float32)[None, :]

    in_maps = []
    for i in range(N_CORES):
        sel = slice(i * B_LOC, (i + 1) * B_LOC)
        xb = x[:, sel, :].transpose(1, 0, 2)              # (B_LOC, S, C)
        xt = np.zeros((B_LOC, C, SP), np.float32)
        xt[:, :, K - 1:] = xb.transpose(0, 2, 1)
        m = {
            "xt_pad": xt,
            "xneg": np.ascontiguousarray(-xb.reshape(B_LOC, M_CHUNKS, P, C)),
            "wconv": wconv_np,
            "linw": linw_np,
            "trid": tri_np,
            "negio": negio_np,
            "desired": (BETA * target_lengths[sel].astype(np.float64) + EPS_CIF)
                        .astype(np.float32).reshape(B_LOC, 1),
        }
        if has_convb:
            m["convb"] = np.ascontiguousarray(conv_b[None, :].astype(np.float32))
        if has_lng:
            m["lngt"] = np.ascontiguousarray(ln_g[None, :].astype(np.float32))
        if has_lnb:
            m["lnbt"] = np.ascontiguousarray(ln_b[None, :].astype(np.float32))
        if has_pmask:
            keep = (~padding_mask[sel]).astype(np.float32)        # (B_LOC, S)
            m["pmaskt"] = np.ascontiguousarray(
                keep.reshape(B_LOC, M_CHUNKS, P).transpose(0, 2, 1))
        in_maps.append(m)
    return in_maps


def _assemble(results):
    cif = np.empty((B, T_MAX, C), np.float32)
    alpha = np.empty((B, S), np.float32)
    for i in range(N_CORES):
        r = results[i]
        co = np.asarray(r["cif_out"]).reshape(B_LOC, T_MAX, C)
        ao = np.asarray(r["alpha_out"])                    # (B_LOC, P, M_CHUNKS)
        for bl in range(B_LOC):
            bg = i * B_LOC + bl
            cif[bg] = co[bl]
            alpha[bg] = ao[bl].T.reshape(S)
    return np.ascontiguousarray(cif.transpose(1, 0, 2)), alpha


def _get_prog(flags):
    if flags not in _prog_cache:
        _prog_cache[flags] = _build(flags)
    return _prog_cache[flags]


def kernel(x, conv_w, conv_b, ln_g, ln_b, lin_w, lin_b, padding_mask,
           target_lengths, _run=None, _mm_r=True, _gelu="hw"):
    x = np.asarray(x, np.float32)
    conv_w = np.asarray(conv_w, np.float32)
    conv_b = np.asarray(conv_b, np.float32)
    ln_g = np.asarray(ln_g, np.float32)
    ln_b = np.asarray(ln_b, np.float32)
    lin_w = np.asarray(lin_w, np.float32)
    lin_b = np.asarray(lin_b, np.float32)
    padding_mask = np.asarray(padding_mask)
    target_lengths = np.asarray(target_lengths)

    flags = (
        bool(_mm_r),
        bool(np.any(conv_b)),
        bool(np.any(ln_g != 1.0)),
        bool(np.any(ln_b)),
        bool(np.any(padding_mask)),
        float(lin_b[0]),
        _gelu,
    )
    nc = _get_prog(flags)
    in_maps = _prep_inputs(x, conv_w, lin_w, target_lengths, conv_b, ln_g,
                           ln_b, padding_mask, flags)
    if _run is not None:                     # test hook (e.g. CoreSim)
        results = _run(nc, in_maps)
    else:
        results = run_bass_kernel_spmd(nc, in_maps,
                                       core_ids=list(range(N_CORES))).results
    return _assemble(results)
